# revision 2
# baseline (speedup 1.0000x reference)
"""GCN (3-layer + head) on 8 TRN2 cores.

Sharding: targets split across cores (12.5k nodes each, no all-reduce of
aggregates needed); within a core, edge streams split by source-chunk into the
8 GpSimd groups. Per layer: linear (PE) -> g = dis*h (DVE) -> AllGather g ->
ap_gather message streams (GpSimd) -> prefix scan (DVE) -> boundary gather
(GpSimd) -> group-fold (PE ones-matmul) -> diff (DVE) -> tanh tail (ACT).
Degrees come from the boundary-position tensor (device-side diff+fold).
"""

from contextlib import ExitStack

import numpy as np

import concourse.bacc as bacc
import concourse.tile as tile
from concourse import mybir

F32 = mybir.dt.float32
I16 = mybir.dt.int16
AOT = mybir.AluOpType
ACTF = mybir.ActivationFunctionType

NC = 8  # cores
NG = 8  # gpsimd groups (source chunks)


def make_cfg(n, PH, SLEN_PH, CALLN):
    NT = n // NC
    assert NT % PH == 0
    NPH = NT // PH
    BPH = ((NPH + 1 + 15) // 16) * 16
    assert SLEN_PH % CALLN == 0 and SLEN_PH % 16 == 0 and CALLN % 4 == 0
    assert SLEN_PH <= 32768 and NT + 1 <= 32768
    return dict(n=n, NT=NT, CH=NT, PH=PH, NPH=NPH, BPH=BPH,
                SLEN_PH=SLEN_PH, CALLN=CALLN)


def host_prep(cfg, x, edge_index, W1, b1, W2, b2, W3, b3, Wc, bc):
    n, NT, CH = cfg["n"], cfg["NT"], cfg["CH"]
    PH, NPH, BPH, SLEN = cfg["PH"], cfg["NPH"], cfg["BPH"], cfg["SLEN_PH"]
    row = np.asarray(edge_index[0], dtype=np.int64)
    col = np.asarray(edge_index[1], dtype=np.int64)
    x = np.asarray(x, dtype=np.float32)

    # lane-packed weights: lane = 4*r + f (4 replicas x 4 features)
    def lanes_in(Wmat):  # [fo, fi] -> [k=34 or 16, m=16]
        fo, fi = Wmat.shape
        out = np.zeros((fi, 16), np.float32)
        for r in range(4):
            out[:, 4 * r : 4 * r + fo] = Wmat.T
        return out

    def lanes_sq(Wmat):  # [fo, fi] -> block-diag per replica [16, 16]
        fo, fi = Wmat.shape
        out = np.zeros((16, 16), np.float32)
        for r in range(4):
            out[4 * r : 4 * r + fi, 4 * r : 4 * r + fo] = Wmat.T
        return out

    w1l = lanes_in(np.asarray(W1, np.float32))  # [34, 16]
    w2l = lanes_sq(np.asarray(W2, np.float32))
    w3l = lanes_sq(np.asarray(W3, np.float32))
    wcl = lanes_sq(np.asarray(Wc, np.float32))
    blanes = np.zeros((16, 4), np.float32)
    for j, b in enumerate([b1, b2, b3, bc]):
        b = np.asarray(b, np.float32)
        for r in range(4):
            blanes[4 * r : 4 * r + len(b), j] = b
    foldw = np.zeros((128, 16), np.float32)
    for k in range(128):
        foldw[k, k % 16] = 1.0

    def interleave16(a2d):  # [rows of 16-wide wraps] stream [G, L] -> [G*16, L//16]
        G, L = a2d.shape
        assert L % 16 == 0
        return a2d.reshape(G, L // 16, 16).transpose(0, 2, 1).reshape(G * 16, L // 16)

    in_maps = []
    for c in range(NC):
        sel = (col // NT) == c
        r_c = row[sel]
        v_c = (col[sel] - c * NT).astype(np.int64)
        g_c = r_c // CH
        rl_c = (r_c % CH).astype(np.int64)
        order = np.argsort(g_c * NT + v_c, kind="stable")
        g_s, v_s, rl_s = g_c[order], v_c[order], rl_c[order]
        cnt = np.bincount(g_s * NT + v_s, minlength=NG * NT).reshape(NG, NT)

        idx_stream = np.full((NG, PH * SLEN), CH, np.int16)
        bpos = np.zeros((NG, PH * BPH), np.float32)
        # per group, slot values in (g, v) order
        gstarts = np.concatenate([[0], np.cumsum(cnt.sum(axis=1))])
        for g in range(NG):
            vals_g = rl_s[gstarts[g] : gstarts[g + 1]]  # sorted by v
            pcnt = cnt[g].reshape(PH, NPH)
            poff = np.concatenate([[0], np.cumsum(pcnt.sum(axis=1))])
            for ph in range(PH):
                vals = vals_g[poff[ph] : poff[ph + 1]]
                assert len(vals) + 1 <= SLEN, f"phase stream overflow {len(vals)}"
                idx_stream[g, ph * SLEN + 1 : ph * SLEN + 1 + len(vals)] = vals
                # boundary positions: B[0]=0 (lead pad); B[j] = last slot of
                # node j-1 = cumsum(cnt)[j-1] (slots start at index 1)
                b = np.zeros(BPH, np.float32)
                b[1 : NPH + 1] = np.cumsum(pcnt[ph]).astype(np.float32)
                bpos[g, ph * BPH : (ph + 1) * BPH] = b
        bidx = bpos.astype(np.int16)  # same positions as int16 gather indices

        idx_t = interleave16(idx_stream.reshape(NG, PH * SLEN)).astype(np.int16)
        bidx_t = interleave16(bidx.reshape(NG, PH * BPH)).astype(np.int16)
        # replicate each group's row across its 16 partitions
        idx_128 = np.repeat(idx_t.reshape(NG, 16, -1), 1, axis=0).reshape(128, -1)
        bidx_128 = bidx_t.reshape(128, -1)
        bpos_128 = np.repeat(bpos[:, None, :], 16, axis=1).reshape(128, PH * BPH)

        xt_own = np.ascontiguousarray(x[c * NT : (c + 1) * NT].T)  # [34, NT]
        in_maps.append(
            dict(
                xt=xt_own,
                idxs=idx_128,
                bidx=bidx_128,
                bpos=bpos_128,
                w1l=w1l, w2l=w2l, w3l=w3l, wcl=wcl,
                blanes=blanes, foldw=foldw,
            )
        )
    return in_maps


def build_nc(cfg):
    n, NT, CH = cfg["n"], cfg["NT"], cfg["CH"]
    PH, NPH, BPH, SLEN, CALLN = (
        cfg["PH"], cfg["NPH"], cfg["BPH"], cfg["SLEN_PH"], cfg["CALLN"],
    )
    nc = bacc.Bacc()
    xt = nc.declare_dram_parameter("xt", [34, NT], F32, isOutput=False)
    idxs = nc.declare_dram_parameter("idxs", [128, PH * SLEN // 16], I16, isOutput=False)
    bidx = nc.declare_dram_parameter("bidx", [128, PH * BPH // 16], I16, isOutput=False)
    bpos = nc.declare_dram_parameter("bpos", [128, PH * BPH], F32, isOutput=False)
    w1l = nc.declare_dram_parameter("w1l", [34, 16], F32, isOutput=False)
    w2l = nc.declare_dram_parameter("w2l", [16, 16], F32, isOutput=False)
    w3l = nc.declare_dram_parameter("w3l", [16, 16], F32, isOutput=False)
    wcl = nc.declare_dram_parameter("wcl", [16, 16], F32, isOutput=False)
    blanes = nc.declare_dram_parameter("blanes", [16, 4], F32, isOutput=False)
    foldw = nc.declare_dram_parameter("foldw", [128, 16], F32, isOutput=False)
    h1o = nc.declare_dram_parameter("h1o", [4, NT], F32, isOutput=True)
    h2o = nc.declare_dram_parameter("h2o", [4, NT], F32, isOutput=True)
    h3o = nc.declare_dram_parameter("h3o", [2, NT], F32, isOutput=True)
    outo = nc.declare_dram_parameter("outo", [4, NT], F32, isOutput=True)

    h_own = nc.dram_tensor("h_own", [16, NT], F32)
    g_own = nc.dram_tensor("g_own", [16, NT], F32)
    dis_d = nc.dram_tensor("dis_d", [16, NT], F32)
    ag = nc.dram_tensor("ag", [128, NT], F32, addr_space="Shared")

    with tile.TileContext(nc) as tc, ExitStack() as ctx:
        const = ctx.enter_context(tc.tile_pool(name="const", bufs=1))
        big = ctx.enter_context(tc.tile_pool(name="big", bufs=1))
        sb = ctx.enter_context(tc.tile_pool(name="sb", bufs=2))
        ps = ctx.enter_context(tc.tile_pool(name="ps", bufs=2, space="PSUM"))

        def dma(dst, src):
            nc.sync.dma_start(out=dst, in_=src)

        t_idx = const.tile([128, PH * SLEN // 16], I16)
        dma(t_idx[:], idxs[:])
        t_bidx = const.tile([128, PH * BPH // 16], I16)
        dma(t_bidx[:], bidx[:])
        t_foldw = const.tile([128, 16], F32)
        dma(t_foldw[:], foldw[:])
        t_w1l = const.tile([34, 16], F32)
        dma(t_w1l[:], w1l[:])
        t_w2l = const.tile([16, 16], F32)
        dma(t_w2l[:], w2l[:])
        t_w3l = const.tile([16, 16], F32)
        dma(t_w3l[:], w3l[:])
        t_wcl = const.tile([16, 16], F32)
        dma(t_wcl[:], wcl[:])
        t_bl = const.tile([16, 4], F32)
        dma(t_bl[:], blanes[:])
        zero1 = const.tile([128, 1], F32)
        nc.vector.memset(zero1[:], 0.0)

        table = big.tile([128, CH + 1], F32)
        stream = big.tile([128, SLEN], F32)
        prefix = big.tile([128, SLEN], F32)

        def fold_diff(src_ap, ncols, out_tile):
            """src [128, >=ncols] -> out [16, ncols-1]: diff(fold_groups(src))."""
            S = sb.tile([16, BPH], F32, tag="S")
            for t0 in range(0, ncols, 512):
                w = min(512, ncols - t0)
                p = ps.tile([16, 512], F32, tag="foldp")
                nc.tensor.matmul(
                    out=p[:, :w], lhsT=t_foldw[:], rhs=src_ap[:, t0 : t0 + w],
                    start=True, stop=True,
                )
                nc.scalar.copy(out=S[:, t0 : t0 + w], in_=p[:, :w])
            nc.vector.tensor_tensor(
                out=out_tile[:], in0=S[:, 1:ncols], in1=S[:, : ncols - 1],
                op=AOT.subtract,
            )

        # ---------- degrees -> dis ----------
        for ph in range(PH):
            bp = sb.tile([128, BPH], F32, tag="bp")
            dma(bp[:], bpos[:, ph * BPH : (ph + 1) * BPH])
            degp = sb.tile([16, NPH], F32, tag="degp")
            fold_diff(bp[:], NPH + 1, degp)
            nc.vector.tensor_scalar_add(out=degp[:], in0=degp[:], scalar1=1.0)
            rec = sb.tile([16, NPH], F32, tag="rec")
            nc.vector.reciprocal(out=rec[:], in_=degp[:])
            disp = sb.tile([16, NPH], F32, tag="disp")
            nc.scalar.activation(out=disp[:], in_=rec[:], func=ACTF.Sqrt)
            dma(dis_d[:, ph * NPH : (ph + 1) * NPH], disp[:])

        # ---------- layers ----------
        for L in (1, 2, 3):
            for ph in range(PH):
                sl = slice(ph * NPH, (ph + 1) * NPH)
                if L == 1:
                    src = sb.tile([34, NPH], F32, tag="xsrc")
                    dma(src[:], xt[:, sl])
                    wl = t_w1l
                else:
                    src = sb.tile([16, NPH], F32, tag="hsrc")
                    dma(src[:], h_own[:, sl])
                    wl = t_w2l if L == 2 else t_w3l
                hw = sb.tile([16, NPH], F32, tag="hw")
                for t0 in range(0, NPH, 512):
                    w = min(512, NPH - t0)
                    p = ps.tile([16, 512], F32, tag="linp")
                    nc.tensor.matmul(
                        out=p[:, :w], lhsT=wl[:], rhs=src[:, t0 : t0 + w],
                        start=True, stop=True,
                    )
                    nc.scalar.copy(out=hw[:, t0 : t0 + w], in_=p[:, :w])
                disp = sb.tile([16, NPH], F32, tag="disg")
                dma(disp[:], dis_d[:, sl])
                gp = sb.tile([16, NPH], F32, tag="gp")
                nc.vector.tensor_tensor(out=gp[:], in0=hw[:], in1=disp[:], op=AOT.mult)
                dma(g_own[:, sl], gp[:])

            nc.gpsimd.collective_compute(
                "AllGather", AOT.bypass,
                replica_groups=[list(range(NC))],
                ins=[g_own[:]], outs=[ag[:]],
            )
            dma(table[:, :CH], ag[:])
            nc.vector.memset(table[:, CH : CH + 1], 0.0)

            for ph in range(PH):
                sl = slice(ph * NPH, (ph + 1) * NPH)
                for o in range(0, SLEN, CALLN):
                    nc.gpsimd.ap_gather(
                        stream[:, o : o + CALLN], table[:],
                        t_idx[:, (ph * SLEN + o) // 16 : (ph * SLEN + o + CALLN) // 16],
                        channels=128, num_elems=CH + 1, d=1, num_idxs=CALLN,
                    )
                nc.vector.tensor_tensor_scan(
                    prefix[:], zero1[:].to_broadcast([128, SLEN]), stream[:],
                    0.0, AOT.add, AOT.add,
                )
                bv = sb.tile([128, BPH], F32, tag="bv")
                nc.gpsimd.ap_gather(
                    bv[:], prefix[:],
                    t_bidx[:, ph * BPH // 16 : (ph + 1) * BPH // 16],
                    channels=128, num_elems=SLEN, d=1, num_idxs=BPH,
                )
                agg = sb.tile([16, NPH], F32, tag="agg")
                fold_diff(bv[:], NPH + 1, agg)
                gp2 = sb.tile([16, NPH], F32, tag="gp2")
                dma(gp2[:], g_own[:, sl])
                disp2 = sb.tile([16, NPH], F32, tag="dis2")
                dma(disp2[:], dis_d[:, sl])
                t1 = sb.tile([16, NPH], F32, tag="t1")
                nc.vector.tensor_tensor(out=t1[:], in0=agg[:], in1=gp2[:], op=AOT.add)
                t2 = sb.tile([16, NPH], F32, tag="t2")
                nc.vector.tensor_tensor(out=t2[:], in0=t1[:], in1=disp2[:], op=AOT.mult)
                hn = sb.tile([16, NPH], F32, tag="hn")
                nc.scalar.activation(
                    out=hn[:], in_=t2[:], func=ACTF.Tanh, bias=t_bl[:, L - 1 : L],
                )
                if L < 3:
                    dma(h_own[:, sl], hn[:])
                if L == 1:
                    dma(h1o[:, sl], hn[0:4, :])
                elif L == 2:
                    dma(h2o[:, sl], hn[0:4, :])
                else:
                    dma(h3o[:, sl], hn[0:2, :])
                    oo = sb.tile([16, NPH], F32, tag="oo")
                    for t0 in range(0, NPH, 512):
                        w = min(512, NPH - t0)
                        p = ps.tile([16, 512], F32, tag="headp")
                        nc.tensor.matmul(
                            out=p[:, :w], lhsT=t_wcl[:], rhs=hn[:, t0 : t0 + w],
                            start=True, stop=True,
                        )
                        nc.scalar.activation(
                            out=oo[:, t0 : t0 + w], in_=p[:, :w],
                            func=ACTF.Identity, bias=t_bl[:, 3:4],
                        )
                    dma(outo[:, sl], oo[0:4, :])
    nc.finalize()
    return nc


def unshard(results, cfg):
    NT = cfg["NT"]
    outs = {k: [] for k in ("outo", "h1o", "h2o", "h3o")}
    for c in range(NC):
        for k in outs:
            outs[k].append(np.ascontiguousarray(results[c][k].T))  # [NT, f]
    return tuple(np.concatenate(outs[k], axis=0) for k in ("outo", "h1o", "h2o", "h3o"))


from concourse.bass_utils import run_bass_kernel_spmd


def kernel(**inputs):
    """Full-input GCN kernel: shards internally across 8 NeuronCores."""
    n = int(np.asarray(inputs["x"]).shape[0])
    # pick phase stream length from the data (rounded to CALLN)
    CALLN = 1024
    PH = 20
    NT = n // NC
    col = np.asarray(inputs["edge_index"][1], dtype=np.int64)
    row = np.asarray(inputs["edge_index"][0], dtype=np.int64)
    NPH = NT // PH
    # max edges per (core, phase, group)
    key = (col // NT) * (PH * NG) + ((col % NT) // NPH) * NG + (row // NT)
    maxlen = int(np.bincount(key, minlength=NC * PH * NG).max())
    SLEN_PH = ((maxlen + 1 + CALLN) // CALLN) * CALLN
    cfg = make_cfg(n, PH=PH, SLEN_PH=SLEN_PH, CALLN=CALLN)
    in_maps = host_prep(cfg, **{k: inputs[k] for k in (
        "x", "edge_index", "W1", "b1", "W2", "b2", "W3", "b3", "Wc", "bc")})
    nc = build_nc(cfg)
    res = run_bass_kernel_spmd(nc, in_maps, list(range(NC))).results
    return unshard(res, cfg)
